# revision 5
# baseline (speedup 1.0000x reference)
"""Multi-head attention (B=2, S=2048, H=1024, 16 heads) on 8 trn2 NeuronCores.

Sharding: tensor-parallel over heads — each core owns 2 heads (128 channels of
the QKV projections and 128 input channels of the output projection). Every
core consumes the full (transposed, bf16-cast) activations; partial outputs of
the wo projection are summed on the host.

Device-side dataflow per core (all matmuls bf16 with f32 PSUM accumulation):
  QT[c,s] = (wq_c x^T + bq) : transposed projections, channels on partitions
  KT[c,s] likewise; V[s,c] in natural layout (tokens on partitions)
  scores^T[k,q] = KT_h^T-tile . QT_h  (two heads row-packed on the PE array)
  E = exp(scores/8)  (no max subtraction: scores are ~N(0,1), |s| < ~6)
  O^T[d,q], sums[q] accumulate over key tiles via ones-augmented V (M=65)
  O_norm = O^T * bcast(1/sums); y^T partial = woT_c . O_norm
"""

import os
import threading

import numpy as np
import ml_dtypes

import concourse.bass as bass
import concourse.mybir as mybir
import concourse.tile as tile
from concourse import bacc
from concourse.bass_utils import run_bass_kernel_spmd

BF16 = ml_dtypes.bfloat16
F32 = mybir.dt.float32
BF = mybir.dt.bfloat16

B = 2
S = 2048
H = 1024
NS = B * S          # 4096 tokens
NH_LOCAL = 2        # heads per core
HD = 64             # head dim
CPC = 128           # channels per core
NF = H // 128       # feature chunks
N_CORES = 8

_cache = threading.Lock()
_nc = None

LAST_RESULT = None  # BassKernelResults of the most recent run (for test.py)


def _build_nc():
    nc = bacc.Bacc(None, target_bir_lowering=False, debug=False)

    xq_d = nc.dram_tensor("xq_t", [H, NS], BF, kind="ExternalInput")
    xk_d = nc.dram_tensor("xk_t", [H, NS], BF, kind="ExternalInput")
    xv_d = nc.dram_tensor("xv_t", [H, NS], BF, kind="ExternalInput")
    wq_d = nc.dram_tensor("wq_t", [H, CPC], BF, kind="ExternalInput")
    wk_d = nc.dram_tensor("wk_t", [H, CPC], BF, kind="ExternalInput")
    wv_d = nc.dram_tensor("wv_t", [H, CPC], BF, kind="ExternalInput")
    bq_d = nc.dram_tensor("bq", [CPC, 1], F32, kind="ExternalInput")
    bk_d = nc.dram_tensor("bk", [CPC, 1], F32, kind="ExternalInput")
    bv_d = nc.dram_tensor("bv", [1, CPC], BF, kind="ExternalInput")
    wo_d = nc.dram_tensor("wo_t", [CPC, H], BF, kind="ExternalInput")
    y_d = nc.dram_tensor("y_t", [H, NS], F32, kind="ExternalOutput")

    xq_ap = xq_d.rearrange("(nf p) s -> nf p s", p=128)
    xk_ap = xk_d.rearrange("(nf p) s -> nf p s", p=128)
    xv_ap = xv_d.rearrange("(nf p) s -> nf p s", p=128)
    y_ap = y_d.rearrange("(no p) s -> no p s", p=128)

    Exp = mybir.ActivationFunctionType.Exp
    Copy = mybir.ActivationFunctionType.Identity

    with tile.TileContext(nc) as tc:
        with (
            tc.tile_pool(name="const", bufs=1) as const,
            tc.tile_pool(name="res", bufs=1) as res,
            tc.tile_pool(name="work", bufs=3) as work,
            tc.tile_pool(name="psum", bufs=1, space="PSUM") as psum,
        ):
            # --- constants / weights ---
            wq_sb = const.tile([128, NF, CPC], BF)
            wk_sb = const.tile([128, NF, CPC], BF)
            wv_sb = const.tile([128, NF, CPC], BF)
            wo_sb = const.tile([128, NF, 128], BF)
            bq_sb = const.tile([128, 1], F32)
            bk_sb = const.tile([128, 1], F32)
            bv_sb = const.tile([1, CPC], BF)
            ones1 = const.tile([1, 128], BF)
            nc.sync.dma_start(wq_sb[:], wq_d.rearrange("(nf p) c -> p nf c", p=128))
            nc.sync.dma_start(wk_sb[:], wk_d.rearrange("(nf p) c -> p nf c", p=128))
            nc.sync.dma_start(wv_sb[:], wv_d.rearrange("(nf p) c -> p nf c", p=128))
            nc.sync.dma_start(wo_sb[:], wo_d.rearrange("p (no c) -> p no c", c=128))
            nc.sync.dma_start(bq_sb[:], bq_d[:])
            nc.sync.dma_start(bk_sb[:], bk_d[:])
            nc.sync.dma_start(bv_sb[:], bv_d[:])
            nc.gpsimd.memset(ones1[:], 1.0)

            # --- residents ---
            QT = res.tile([128, NS], BF)
            KT = res.tile([128, NS], BF)
            V0 = res.tile([128, NS // 128, HD + 1], BF)
            V1 = res.tile([128, NS // 128, HD + 1], BF)
            nc.gpsimd.memset(V0[:, :, HD : HD + 1], 1.0)
            nc.gpsimd.memset(V1[:, :, HD : HD + 1], 1.0)

            # --- projections ---
            with tc.tile_pool(name="xin", bufs=10) as xin:
                for name, x_ap, w_sb, b_sb, out_t in (
                    ("q", xq_ap, wq_sb, bq_sb, QT),
                    ("k", xk_ap, wk_sb, bk_sb, KT),
                ):
                    xt = []
                    for f in range(NF):
                        t = xin.tile([128, NS], BF, tag="xc", name=f"x{name}{f}")
                        nc.sync.dma_start(t[:], x_ap[f])
                        xt.append(t)
                    for sw in range(NS // 512):
                        ps = psum.tile([128, 512], F32, tag="s", name=f"ps{name}{sw}")
                        for f in range(NF):
                            nc.tensor.matmul(
                                ps[:],
                                lhsT=w_sb[:, f, :],
                                rhs=xt[f][:, sw * 512 : (sw + 1) * 512],
                                start=(f == 0),
                                stop=(f == NF - 1),
                            )
                        nc.scalar.activation(
                            out_t[:, sw * 512 : (sw + 1) * 512], ps[:], Copy,
                            bias=b_sb[:],
                        )
                # V (natural layout, tokens on partitions)
                xtv = []
                for f in range(NF):
                    t = xin.tile([128, NS], BF, tag="xc", name=f"xv{f}")
                    nc.sync.dma_start(t[:], xv_ap[f])
                    xtv.append(t)
                for si in range(NS // 128):
                    psv = psum.tile([128, 128], F32, tag="s", name=f"psv{si}")
                    for f in range(NF):
                        nc.tensor.matmul(
                            psv[:],
                            lhsT=xtv[f][:, si * 128 : (si + 1) * 128],
                            rhs=wv_sb[:, f, :],
                            start=(f == 0),
                            stop=False,
                        )
                    nc.tensor.matmul(
                        psv[:], lhsT=ones1[:], rhs=bv_sb[:], start=False, stop=True
                    )
                    nc.vector.tensor_copy(V0[:, si, 0:HD], psv[:, 0:HD])
                    nc.vector.tensor_copy(V1[:, si, 0:HD], psv[:, HD:128])

            # --- attention + output projection ---
            with (
                tc.tile_pool(name="epool", bufs=4) as epool,
                tc.tile_pool(name="npool", bufs=2) as npool,
                tc.tile_pool(name="ypool", bufs=3) as ypool,
                tc.tile_pool(name="opsum", bufs=2, space="PSUM") as opsum,
            ):
                for b in range(B):
                    for qw in range(2):
                        q0 = b * S + qw * 1024
                        po0 = opsum.tile([65, 1024], F32, tag="o", name=f"po0_{b}{qw}")
                        po1 = opsum.tile([65, 1024], F32, tag="o", name=f"po1_{b}{qw}")
                        for k2t in range(S // 128):
                            si = b * 16 + k2t
                            ks = si * 128
                            ps0 = psum.tile([128, 1024], F32, tag="s",
                                            name=f"ps0_{b}{qw}{k2t}")
                            ps1 = psum.tile([128, 1024], F32, tag="s",
                                            name=f"ps1_{b}{qw}{k2t}")
                            for hf in range(2):
                                qs = q0 + hf * 512
                                fs = slice(hf * 512, (hf + 1) * 512)
                                nc.tensor.matmul(
                                    ps0[:, fs],
                                    lhsT=KT[0:64, ks : ks + 128],
                                    rhs=QT[0:64, qs : qs + 512],
                                    tile_position=(0, 0),
                                )
                                nc.tensor.matmul(
                                    ps1[:, fs],
                                    lhsT=KT[64:128, ks : ks + 128],
                                    rhs=QT[64:128, qs : qs + 512],
                                    tile_position=(64, 0),
                                )
                            e0 = epool.tile([128, 1024], BF, tag="e",
                                            name=f"e0_{b}{qw}{k2t}")
                            e1 = epool.tile([128, 1024], BF, tag="e",
                                            name=f"e1_{b}{qw}{k2t}")
                            nc.scalar.activation(e0[:], ps0[:], Exp, scale=0.125)
                            nc.scalar.activation(e1[:], ps1[:], Exp, scale=0.125)
                            for hf in range(2):
                                fs = slice(hf * 512, (hf + 1) * 512)
                                nc.tensor.matmul(
                                    po0[:, fs], lhsT=V0[:, si, :], rhs=e0[:, fs],
                                    start=(k2t == 0), stop=(k2t == 15),
                                )
                                nc.tensor.matmul(
                                    po1[:, fs], lhsT=V1[:, si, :], rhs=e1[:, fs],
                                    start=(k2t == 0), stop=(k2t == 15),
                                )
                        # normalize: On[hd, q] = O^T[hd, q] / sums[q]
                        # lane-aligned reciprocal (row 64 -> row 64); gpsimd
                        # broadcast handles the partition shift afterwards
                        r0 = npool.tile([65, 1024], F32, tag="r0", name=f"r0_{b}{qw}")
                        r1 = npool.tile([65, 1024], F32, tag="r1", name=f"r1_{b}{qw}")
                        nc.vector.reciprocal(r0[64:65, :], po0[64:65, :])
                        nc.vector.reciprocal(r1[64:65, :], po1[64:65, :])
                        # partition_broadcast only reads base-partition-0 APs;
                        # DMA shifts the row down first
                        rs0 = npool.tile([1, 1024], F32, tag="rs0", name=f"rs0_{b}{qw}")
                        rs1 = npool.tile([1, 1024], F32, tag="rs1", name=f"rs1_{b}{qw}")
                        nc.sync.dma_start(rs0[:], r0[64:65, :])
                        nc.sync.dma_start(rs1[:], r1[64:65, :])
                        rb0 = npool.tile([64, 1024], F32, tag="rb0", name=f"rb0_{b}{qw}")
                        rb1 = npool.tile([64, 1024], F32, tag="rb1", name=f"rb1_{b}{qw}")
                        nc.gpsimd.partition_broadcast(rb0[:], rs0[:])
                        nc.gpsimd.partition_broadcast(rb1[:], rs1[:])
                        on = npool.tile([128, 1024], BF, tag="on", name=f"on_{b}{qw}")
                        on1 = npool.tile([64, 1024], BF, tag="on1", name=f"on1_{b}{qw}")
                        nc.vector.tensor_mul(on[0:64, :], po0[0:64, :], rb0[:])
                        nc.vector.tensor_mul(on1[:], po1[0:64, :], rb1[:])
                        nc.sync.dma_start(on[64:128, :], on1[:])
                        for oc in range(NF):
                            for hf in range(2):
                                fs = slice(hf * 512, (hf + 1) * 512)
                                py = psum.tile([128, 512], F32, tag="s",
                                               name=f"py_{b}{qw}{oc}{hf}")
                                nc.tensor.matmul(
                                    py[:], lhsT=wo_sb[:, oc, :], rhs=on[:, fs]
                                )
                                ysb = ypool.tile([128, 512], F32, tag="y",
                                                 name=f"y_{b}{qw}{oc}{hf}")
                                nc.vector.tensor_copy(ysb[:], py[:])
                                nc.sync.dma_start(
                                    y_ap[oc, :, q0 + hf * 512 : q0 + (hf + 1) * 512],
                                    ysb[:],
                                )
    nc.compile()
    return nc


def _get_nc():
    global _nc
    with _cache:
        if _nc is None:
            _nc = _build_nc()
        return _nc


def kernel(q, k, v, wq_w, wq_b, wk_w, wk_b, wv_w, wv_b, wo_w, wo_b):
    global LAST_RESULT
    nc = _get_nc()

    def xT(a):
        return np.ascontiguousarray(np.asarray(a).reshape(NS, H).astype(BF16).T)

    xq_t, xk_t, xv_t = xT(q), xT(k), xT(v)
    wq_w = np.asarray(wq_w, dtype=np.float32)
    wk_w = np.asarray(wk_w, dtype=np.float32)
    wv_w = np.asarray(wv_w, dtype=np.float32)
    wo_w = np.asarray(wo_w, dtype=np.float32)

    in_maps = []
    for c in range(N_CORES):
        cs = slice(c * CPC, (c + 1) * CPC)
        in_maps.append({
            "xq_t": xq_t,
            "xk_t": xk_t,
            "xv_t": xv_t,
            "wq_t": np.ascontiguousarray(wq_w[cs, :].astype(BF16).T),
            "wk_t": np.ascontiguousarray(wk_w[cs, :].astype(BF16).T),
            "wv_t": np.ascontiguousarray(wv_w[cs, :].astype(BF16).T),
            "bq": np.asarray(wq_b, np.float32)[cs].reshape(CPC, 1),
            "bk": np.asarray(wk_b, np.float32)[cs].reshape(CPC, 1),
            "bv": np.asarray(wv_b, np.float32)[cs].astype(BF16).reshape(1, CPC),
            "wo_t": np.ascontiguousarray(wo_w[:, cs].astype(BF16).T),
        })

    res = run_bass_kernel_spmd(
        nc, in_maps, core_ids=list(range(N_CORES)),
        trace=bool(int(os.environ.get("MHA_TRACE", "0"))),
    )
    LAST_RESULT = res

    y = res.results[0]["y_t"].astype(np.float64)
    for c in range(1, N_CORES):
        y += res.results[c]["y_t"]
    y = y.T + np.asarray(wo_b, np.float64)[None, :]
    return y.reshape(B, S, H).astype(np.float32)


# revision 6
# speedup vs baseline: 1.3832x; 1.3832x over previous
"""Multi-head attention (B=2, S=2048, H=1024, 16 heads) on 8 trn2 NeuronCores.

Sharding: tensor-parallel over heads — each core owns 2 heads (128 channels of
the QKV projections and 128 input channels of the output projection). Every
core consumes the full (transposed, bf16-cast) activations; partial outputs of
the wo projection are summed on the host.

Device-side dataflow per core (all matmuls bf16 with f32 PSUM accumulation):
  QT[c,s] = (wq_c x^T + bq) : transposed projections, channels on partitions
  KT[c,s] likewise; V[s,c] in natural layout (tokens on partitions)
  scores^T[k,q] = KT_h^T-tile . QT_h  (two heads row-packed on the PE array)
  E = exp(scores/8)  (no max subtraction: scores are ~N(0,1), |s| < ~6)
  O^T[d,q], sums[q] accumulate over key tiles via ones-augmented V (M=65)
  O_norm = O^T * bcast(1/sums); y^T partial = woT_c . O_norm
"""

import os
import threading

import numpy as np
import ml_dtypes

import concourse.bass as bass
import concourse.mybir as mybir
import concourse.tile as tile
from concourse import bacc
from concourse.bass_utils import run_bass_kernel_spmd

BF16 = ml_dtypes.bfloat16
F32 = mybir.dt.float32
BF = mybir.dt.bfloat16

B = 2
S = 2048
H = 1024
NS = B * S          # 4096 tokens
NH_LOCAL = 2        # heads per core
HD = 64             # head dim
CPC = 128           # channels per core
NF = H // 128       # feature chunks
N_CORES = 8

_cache = threading.Lock()
_nc = None

LAST_RESULT = None  # BassKernelResults of the most recent run (for test.py)


def _build_nc():
    nc = bacc.Bacc(None, target_bir_lowering=False, debug=False)

    xq_d = nc.dram_tensor("xq_t", [H, NS], BF, kind="ExternalInput")
    xk_d = nc.dram_tensor("xk_t", [H, NS], BF, kind="ExternalInput")
    xv_d = nc.dram_tensor("xv_t", [H, NS], BF, kind="ExternalInput")
    wq_d = nc.dram_tensor("wq_t", [H, CPC], BF, kind="ExternalInput")
    wk_d = nc.dram_tensor("wk_t", [H, CPC], BF, kind="ExternalInput")
    wv_d = nc.dram_tensor("wv_t", [H, CPC], BF, kind="ExternalInput")
    bq_d = nc.dram_tensor("bq", [CPC, 1], F32, kind="ExternalInput")
    bk_d = nc.dram_tensor("bk", [CPC, 1], F32, kind="ExternalInput")
    bv_d = nc.dram_tensor("bv", [1, CPC], BF, kind="ExternalInput")
    wo_d = nc.dram_tensor("wo_t", [CPC, H], BF, kind="ExternalInput")
    y_d = nc.dram_tensor("y_t", [H, NS], F32, kind="ExternalOutput")

    xq_ap = xq_d.rearrange("(nf p) s -> nf p s", p=128)
    xk_ap = xk_d.rearrange("(nf p) s -> nf p s", p=128)
    xv_ap = xv_d.rearrange("(nf p) s -> nf p s", p=128)
    y_ap = y_d.rearrange("(no p) s -> no p s", p=128)

    Exp = mybir.ActivationFunctionType.Exp
    Copy = mybir.ActivationFunctionType.Identity

    with tile.TileContext(nc) as tc:
        with (
            tc.tile_pool(name="const", bufs=1) as const,
            tc.tile_pool(name="res", bufs=1) as res,
            tc.tile_pool(name="work", bufs=3) as work,
            tc.tile_pool(name="psum", bufs=2, space="PSUM") as psum,
        ):
            # --- constants / weights ---
            wq_sb = const.tile([128, NF, CPC], BF)
            wk_sb = const.tile([128, NF, CPC], BF)
            wv_sb = const.tile([128, NF, CPC], BF)
            wo_sb = const.tile([128, NF, 128], BF)
            bq_sb = const.tile([128, 1], F32)
            bk_sb = const.tile([128, 1], F32)
            bv_sb = const.tile([1, CPC], BF)
            ones1 = const.tile([1, 128], BF)
            nc.sync.dma_start(wq_sb[:], wq_d.rearrange("(nf p) c -> p nf c", p=128))
            nc.sync.dma_start(wk_sb[:], wk_d.rearrange("(nf p) c -> p nf c", p=128))
            nc.sync.dma_start(wv_sb[:], wv_d.rearrange("(nf p) c -> p nf c", p=128))
            nc.sync.dma_start(wo_sb[:], wo_d.rearrange("p (no c) -> p no c", c=128))
            nc.sync.dma_start(bq_sb[:], bq_d[:])
            nc.sync.dma_start(bk_sb[:], bk_d[:])
            nc.sync.dma_start(bv_sb[:], bv_d[:])
            nc.gpsimd.memset(ones1[:], 1.0)

            # --- residents ---
            QT = res.tile([128, NS], BF)
            KT = res.tile([128, NS], BF)
            V0 = res.tile([128, NS // 128, HD + 1], BF)
            V1 = res.tile([128, NS // 128, HD + 1], BF)
            nc.gpsimd.memset(V0[:, :, HD : HD + 1], 1.0)
            nc.gpsimd.memset(V1[:, :, HD : HD + 1], 1.0)

            # --- projections ---
            with tc.tile_pool(name="xin", bufs=10) as xin:
                for name, x_ap, w_sb, b_sb, out_t in (
                    ("q", xq_ap, wq_sb, bq_sb, QT),
                    ("k", xk_ap, wk_sb, bk_sb, KT),
                ):
                    xt = []
                    for f in range(NF):
                        t = xin.tile([128, NS], BF, tag="xc", name=f"x{name}{f}")
                        nc.sync.dma_start(t[:], x_ap[f])
                        xt.append(t)
                    for sw in range(NS // 512):
                        ps = psum.tile([128, 512], F32, tag="s", name=f"ps{name}{sw}")
                        for f in range(NF):
                            nc.tensor.matmul(
                                ps[:],
                                lhsT=w_sb[:, f, :],
                                rhs=xt[f][:, sw * 512 : (sw + 1) * 512],
                                start=(f == 0),
                                stop=(f == NF - 1),
                            )
                        nc.scalar.activation(
                            out_t[:, sw * 512 : (sw + 1) * 512], ps[:], Copy,
                            bias=b_sb[:],
                        )
                # V (natural layout, tokens on partitions)
                xtv = []
                for f in range(NF):
                    t = xin.tile([128, NS], BF, tag="xc", name=f"xv{f}")
                    nc.sync.dma_start(t[:], xv_ap[f])
                    xtv.append(t)
                for si in range(NS // 128):
                    psv = psum.tile([128, 128], F32, tag="s", name=f"psv{si}")
                    for f in range(NF):
                        nc.tensor.matmul(
                            psv[:],
                            lhsT=xtv[f][:, si * 128 : (si + 1) * 128],
                            rhs=wv_sb[:, f, :],
                            start=(f == 0),
                            stop=False,
                        )
                    nc.tensor.matmul(
                        psv[:], lhsT=ones1[:], rhs=bv_sb[:], start=False, stop=True
                    )
                    nc.vector.tensor_copy(V0[:, si, 0:HD], psv[:, 0:HD])
                    nc.vector.tensor_copy(V1[:, si, 0:HD], psv[:, HD:128])

            # --- attention + output projection ---
            with (
                tc.tile_pool(name="epool", bufs=6) as epool,
                tc.tile_pool(name="npool", bufs=2) as npool,
                tc.tile_pool(name="ypool", bufs=3) as ypool,
                tc.tile_pool(name="opsum", bufs=2, space="PSUM") as opsum,
            ):
                for b in range(B):
                    for qw in range(2):
                        q0 = b * S + qw * 1024
                        po0 = opsum.tile([65, 1024], F32, tag="o", name=f"po0_{b}{qw}")
                        po1 = opsum.tile([65, 1024], F32, tag="o", name=f"po1_{b}{qw}")
                        for k2t in range(S // 128):
                            si = b * 16 + k2t
                            ks = si * 128
                            ps0 = psum.tile([128, 1024], F32, tag="s",
                                            name=f"ps0_{b}{qw}{k2t}")
                            ps1 = psum.tile([128, 1024], F32, tag="s",
                                            name=f"ps1_{b}{qw}{k2t}")
                            for hf in range(2):
                                qs = q0 + hf * 512
                                fs = slice(hf * 512, (hf + 1) * 512)
                                nc.tensor.matmul(
                                    ps0[:, fs],
                                    lhsT=KT[0:64, ks : ks + 128],
                                    rhs=QT[0:64, qs : qs + 512],
                                    tile_position=(0, 0),
                                )
                                nc.tensor.matmul(
                                    ps1[:, fs],
                                    lhsT=KT[64:128, ks : ks + 128],
                                    rhs=QT[64:128, qs : qs + 512],
                                    tile_position=(64, 0),
                                )
                            e0 = epool.tile([128, 1024], BF, tag="e",
                                            name=f"e0_{b}{qw}{k2t}")
                            e1 = epool.tile([128, 1024], BF, tag="e",
                                            name=f"e1_{b}{qw}{k2t}")
                            nc.scalar.activation(e0[:], ps0[:], Exp, scale=0.125)
                            nc.scalar.activation(e1[:], ps1[:], Exp, scale=0.125)
                            for hf in range(2):
                                fs = slice(hf * 512, (hf + 1) * 512)
                                nc.tensor.matmul(
                                    po0[:, fs], lhsT=V0[:, si, :], rhs=e0[:, fs],
                                    start=(k2t == 0), stop=(k2t == 15),
                                )
                                nc.tensor.matmul(
                                    po1[:, fs], lhsT=V1[:, si, :], rhs=e1[:, fs],
                                    start=(k2t == 0), stop=(k2t == 15),
                                )
                        # normalize: On[hd, q] = O^T[hd, q] / sums[q]
                        # lane-aligned reciprocal (row 64 -> row 64); gpsimd
                        # broadcast handles the partition shift afterwards
                        r0 = npool.tile([65, 1024], F32, tag="r0", name=f"r0_{b}{qw}")
                        r1 = npool.tile([65, 1024], F32, tag="r1", name=f"r1_{b}{qw}")
                        nc.vector.reciprocal(r0[64:65, :], po0[64:65, :])
                        nc.vector.reciprocal(r1[64:65, :], po1[64:65, :])
                        # partition_broadcast only reads base-partition-0 APs;
                        # DMA shifts the row down first
                        rs0 = npool.tile([1, 1024], F32, tag="rs0", name=f"rs0_{b}{qw}")
                        rs1 = npool.tile([1, 1024], F32, tag="rs1", name=f"rs1_{b}{qw}")
                        nc.scalar.dma_start(rs0[:], r0[64:65, :])
                        nc.scalar.dma_start(rs1[:], r1[64:65, :])
                        rb0 = npool.tile([64, 1024], F32, tag="rb0", name=f"rb0_{b}{qw}")
                        rb1 = npool.tile([64, 1024], F32, tag="rb1", name=f"rb1_{b}{qw}")
                        nc.gpsimd.partition_broadcast(rb0[:], rs0[:])
                        nc.gpsimd.partition_broadcast(rb1[:], rs1[:])
                        on = npool.tile([128, 1024], BF, tag="on", name=f"on_{b}{qw}")
                        on1 = npool.tile([64, 1024], BF, tag="on1", name=f"on1_{b}{qw}")
                        nc.vector.tensor_mul(on[0:64, :], po0[0:64, :], rb0[:])
                        nc.vector.tensor_mul(on1[:], po1[0:64, :], rb1[:])
                        nc.scalar.dma_start(on[64:128, :], on1[:])
                        for oc in range(NF):
                            for hf in range(2):
                                fs = slice(hf * 512, (hf + 1) * 512)
                                py = psum.tile([128, 512], F32, tag="s",
                                               name=f"py_{b}{qw}{oc}{hf}")
                                nc.tensor.matmul(
                                    py[:], lhsT=wo_sb[:, oc, :], rhs=on[:, fs]
                                )
                                ysb = ypool.tile([128, 512], F32, tag="y",
                                                 name=f"y_{b}{qw}{oc}{hf}")
                                nc.vector.tensor_copy(ysb[:], py[:])
                                nc.sync.dma_start(
                                    y_ap[oc, :, q0 + hf * 512 : q0 + (hf + 1) * 512],
                                    ysb[:],
                                )
    nc.compile()
    return nc


def _get_nc():
    global _nc
    with _cache:
        if _nc is None:
            _nc = _build_nc()
        return _nc


def kernel(q, k, v, wq_w, wq_b, wk_w, wk_b, wv_w, wv_b, wo_w, wo_b):
    global LAST_RESULT
    nc = _get_nc()

    def xT(a):
        return np.ascontiguousarray(np.asarray(a).reshape(NS, H).astype(BF16).T)

    xq_t, xk_t, xv_t = xT(q), xT(k), xT(v)
    wq_w = np.asarray(wq_w, dtype=np.float32)
    wk_w = np.asarray(wk_w, dtype=np.float32)
    wv_w = np.asarray(wv_w, dtype=np.float32)
    wo_w = np.asarray(wo_w, dtype=np.float32)

    in_maps = []
    for c in range(N_CORES):
        cs = slice(c * CPC, (c + 1) * CPC)
        in_maps.append({
            "xq_t": xq_t,
            "xk_t": xk_t,
            "xv_t": xv_t,
            "wq_t": np.ascontiguousarray(wq_w[cs, :].astype(BF16).T),
            "wk_t": np.ascontiguousarray(wk_w[cs, :].astype(BF16).T),
            "wv_t": np.ascontiguousarray(wv_w[cs, :].astype(BF16).T),
            "bq": np.asarray(wq_b, np.float32)[cs].reshape(CPC, 1),
            "bk": np.asarray(wk_b, np.float32)[cs].reshape(CPC, 1),
            "bv": np.asarray(wv_b, np.float32)[cs].astype(BF16).reshape(1, CPC),
            "wo_t": np.ascontiguousarray(wo_w[:, cs].astype(BF16).T),
        })

    res = run_bass_kernel_spmd(
        nc, in_maps, core_ids=list(range(N_CORES)),
        trace=bool(int(os.environ.get("MHA_TRACE", "0"))),
    )
    LAST_RESULT = res

    y = res.results[0]["y_t"].astype(np.float64)
    for c in range(1, N_CORES):
        y += res.results[c]["y_t"]
    y = y.T + np.asarray(wo_b, np.float64)[None, :]
    return y.reshape(B, S, H).astype(np.float32)
